# revision 1
# baseline (speedup 1.0000x reference)
# Dense-MoE (all experts active) Trainium2 kernel, expert-parallel over 8
# NeuronCores. Each core computes its expert's 2-layer MLP over all tokens:
#   fe_e = gelu(h @ W1[e] + b1[e]) @ (probs[e] * W2[e]) + probs[e] * b2[e]
# then chunked ReduceScatter(add) across the 8 cores sums the expert
# contributions; the host reassembles the full [B, D] output.
#
# Layout: activations are kept transposed on-chip.
#   hT   [IN, Btile]  (PE transpose of h tiles)
#   hidT [H, Btile] = W1_block.T @ hT per 128-row chunk, gelu+b1 via ACT
#   fe   [Btile, D] = hidT_chunk.T @ W2_chunk accumulated over H chunks
# Matmuls run in float16 by default (full PE rate, ~4e-4 rel err);
# MOE_MM_DTYPE=float32r gives a ~2e-4, ~30% slower fallback.
import os
import sys

sys.path.insert(0, "/opt/trn_rl_repo")

import numpy as np

import concourse.mybir as mybir
from concourse import bacc, tile
from concourse import masks

B, E, IN, H, D = 4096, 8, 1024, 2048, 1024
NCORES = 8
P = 128
BT = 512                  # tokens per B-tile
NBT = B // BT             # 8 B-tiles
NSUB = BT // P            # 4 token sub-tiles per B-tile
KC1 = IN // P             # 8 contraction chunks, layer 1
MC1 = H // P              # 16 H chunks
ND = D // 512             # 2 output column slices of 512
HALF = 2 * P              # 256 rows per ReduceScatter chunk (1 MB)
RS_ROWS = HALF // NCORES  # 32 rows each core receives per RS chunk
NCHUNK = NBT * 2          # 16 RS chunks

F32 = mybir.dt.float32

_CACHE = {}


def build(mm_dtype_name="float16", nbt=NBT, use_collective=True):
    mm_dt = getattr(mybir.dt, mm_dtype_name)
    bf16 = mybir.dt.size(mm_dt) == 2  # 2-byte path: bf16 or fp16
    nc = bacc.Bacc("TRN2", target_bir_lowering=False)

    if bf16:
        hT = nc.declare_dram_parameter("ht", [IN, nbt * BT], mm_dt, isOutput=False)
    else:
        h = nc.declare_dram_parameter("h", [nbt * BT, IN], F32, isOutput=False)
    w1 = nc.declare_dram_parameter("w1", [IN, H], mm_dt, isOutput=False)
    b1t = nc.declare_dram_parameter("b1t", [P, MC1], F32, isOutput=False)
    w2 = nc.declare_dram_parameter("w2", [H, D], mm_dt, isOutput=False)
    b2b = nc.declare_dram_parameter("b2b", [P, D], F32, isOutput=False)
    out_rows = nbt * BT // NCORES if use_collective else nbt * BT
    out = nc.declare_dram_parameter("out", [out_rows, D], F32, isOutput=True)

    with tile.TileContext(nc) as tc:
        with (
            tc.tile_pool(name="weights", bufs=1) as wpool,
            tc.tile_pool(name="consts", bufs=1) as cpool,
            tc.tile_pool(name="hraw", bufs=2) as hraw_pool,
            tc.tile_pool(name="ht", bufs=(3 if mybir.dt.size(mm_dt) == 2 else 2)) as ht_pool,
            tc.tile_pool(name="hid", bufs=(2 if mybir.dt.size(mm_dt) == 2 else 1)) as hid_pool,
            tc.tile_pool(name="fe", bufs=(2 if mybir.dt.size(mm_dt) == 2 else 1)) as fe_pool,
            tc.tile_pool(name="tp_ps", bufs=(1 if bf16 else 2),
                         space="PSUM") as tp_psum,
            tc.tile_pool(name="l1_ps", bufs=(3 if bf16 else 2),
                         space="PSUM") as l1_psum,
            tc.tile_pool(name="l2_ps", bufs=4, space="PSUM") as l2_psum,
            tc.tile_pool(name="dram", bufs=4, space="DRAM") as dram_pool,
        ):
            hr_pre = []
            ht0 = None
            if bf16:
                ht0 = ht_pool.tile([P, KC1 * BT], mm_dt, tag="ht")
            if not bf16:
                ident = cpool.tile([P, P], F32, tag="ident")
                masks.make_identity(nc, ident[:])

                # prefetch the first two h sub-tiles ahead of the weight slabs
                # so the transpose pipeline starts immediately
                def prefetch_hr(s):
                    hr = hraw_pool.tile([P, IN], F32, tag="hr")
                    nc.sync.dma_start(hr[:], h[s * P:(s + 1) * P, :])
                    hr_pre.append(hr)

                prefetch_hr(0)
                prefetch_hr(1)

            # per-slab weight tiles so the first matmuls depend only on their
            # own slab's DMA; first-tile h loads interleaved so the transpose
            # pipeline is never starved behind 16MB of weights
            w1_sb = []
            for k in range(KC1):
                if bf16:
                    # pair each W1 slab with the matching first-tile hT chunk
                    # so L1's k-accumulation can start as soon as pair 0 lands
                    nc.sync.dma_start(
                        ht0[:, k * BT:(k + 1) * BT],
                        hT[k * P:(k + 1) * P, 0:BT],
                    )
                t_ = wpool.tile([P, H], mm_dt, tag=f"w1_{k}")
                nc.sync.dma_start(t_[:], w1[k * P:(k + 1) * P, :])
                w1_sb.append(t_)
                if k == 3 and not bf16:
                    prefetch_hr(2)
            if not bf16:
                prefetch_hr(3)
            b1_sb = cpool.tile([P, MC1], F32, tag="b1")
            nc.sync.dma_start(b1_sb[:], b1t[:])
            w2_sb = []
            for m in range(MC1):
                t_ = wpool.tile([P, D], mm_dt, tag=f"w2_{m}")
                nc.sync.dma_start(t_[:], w2[m * P:(m + 1) * P, :])
                w2_sb.append(t_)
            b2_sb = cpool.tile([P, D], F32, tag="b2")
            nc.sync.dma_start(b2_sb[:], b2b[:])

            for t in range(nbt):
                # --- transpose this tile's h rows into hT ---
                # ht chunk k (IN rows k*128..) lives at columns [k*BT, (k+1)*BT)
                if bf16 and t == 0:
                    ht = ht0
                else:
                    ht = ht_pool.tile([P, KC1 * BT], mm_dt, tag="ht")
                if bf16 and t > 0:
                    # hT comes pre-transposed from the host: plain strided DMA
                    for k in range(KC1):
                        nc.sync.dma_start(
                            ht[:, k * BT:(k + 1) * BT],
                            hT[k * P:(k + 1) * P, t * BT:(t + 1) * BT],
                        )
                elif not bf16:
                    for s in range(NSUB):
                        if t == 0:
                            hr = hr_pre[s]
                        else:
                            hr = hraw_pool.tile([P, IN], F32, tag="hr")
                            nc.sync.dma_start(
                                hr[:], h[t * BT + s * P: t * BT + (s + 1) * P, :]
                            )
                        for k in range(KC1):
                            tp = tp_psum.tile([P, P], F32, tag="tp")
                            nc.tensor.transpose(
                                tp[:], hr[:, k * P:(k + 1) * P], ident[:]
                            )
                            nc.vector.tensor_copy(
                                ht[:, k * BT + s * P: k * BT + (s + 1) * P], tp[:]
                            )

                # --- layer 1: hidT chunk m = (W1 block).T @ hT, + b1, gelu ---
                # hid is split in two halves so layer 2's sweep releases the
                # first half early for the next tile's evictions
                hidA = hid_pool.tile([P, (MC1 // 2) * BT], mm_dt, tag="hidA")
                hidB = hid_pool.tile([P, (MC1 // 2) * BT], mm_dt, tag="hidB")

                def hid_slice(m, lo, hi):
                    half_t = hidA if m < MC1 // 2 else hidB
                    mm_ = m % (MC1 // 2)
                    return half_t[:, mm_ * BT + lo: mm_ * BT + hi]

                for m in range(MC1):
                    ps = l1_psum.tile([P, BT], F32, tag="l1")
                    for k in range(KC1):
                        nc.tensor.matmul(
                            ps[:],
                            w1_sb[k][:, m * P:(m + 1) * P],
                            ht[:, k * BT:(k + 1) * BT],
                            start=(k == 0),
                            stop=(k == KC1 - 1),
                        )
                    nc.scalar.activation(
                        hid_slice(m, 0, BT),
                        ps[:],
                        mybir.ActivationFunctionType.Gelu,
                        bias=b1_sb[:, m:m + 1],
                        scale=1.0,
                    )

                # --- layer 2 + chunked ReduceScatter (2MB per tile,
                # tapering to 2x1MB on the final tile for a short tail) ---
                nhalves = 2 if t == nbt - 1 else 1
                subs_per_chunk = NSUB // nhalves
                for half in range(nhalves):
                    fe_chunk = dram_pool.tile(
                        [subs_per_chunk * P, D], F32, tag="fe_dram"
                    )
                    for si in range(subs_per_chunk):
                        s = half * subs_per_chunk + si
                        # both d-slices accumulate together: the second matmul
                        # of each pair reuses the stationary hid block already
                        # in the PE array (ldweights=False) instead of
                        # reloading it
                        ps_a = l2_psum.tile([P, 512], F32, tag="l2")
                        ps_b = l2_psum.tile([P, 512], F32, tag="l2")
                        pss = [ps_a, ps_b]
                        for m in range(MC1):
                            hs = hid_slice(m, s * P, (s + 1) * P)
                            for d in range(ND):
                                mi = nc.tensor.matmul(
                                    pss[d][:],
                                    hs,
                                    w2_sb[m][:, d * 512:(d + 1) * 512],
                                    start=(m == 0),
                                    stop=(m == MC1 - 1),
                                )
                                if d > 0:
                                    mi.ins.ldweights = False
                        for d in range(ND):
                            fe_sb = fe_pool.tile([P, 512], F32, tag="fe_sb")
                            nc.vector.tensor_add(
                                fe_sb[:], pss[d][:],
                                b2_sb[:, d * 512:(d + 1) * 512]
                            )
                            nc.sync.dma_start(
                                fe_chunk[si * P:(si + 1) * P,
                                         d * 512:(d + 1) * 512],
                                fe_sb[:],
                            )

                    chunk_rows = subs_per_chunk * P // NCORES
                    row0 = (t * BT + half * subs_per_chunk * P) // NCORES
                    if use_collective:
                        rs_chunk = dram_pool.tile(
                            [chunk_rows, D], F32, tag="rs_dram"
                        )
                        nc.gpsimd.collective_compute(
                            "ReduceScatter",
                            mybir.AluOpType.add,
                            replica_groups=[list(range(NCORES))],
                            ins=[fe_chunk[:]],
                            outs=[rs_chunk[:]],
                        )
                        nc.sync.dma_start(
                            out[row0:row0 + chunk_rows, :], rs_chunk[:]
                        )
                    else:
                        r0 = t * BT + half * subs_per_chunk * P
                        nc.sync.dma_start(
                            out[r0:r0 + subs_per_chunk * P, :], fe_chunk[:]
                        )

    nc.finalize()
    return nc


def _get_nc(mm_dtype_name):
    key = mm_dtype_name
    if key not in _CACHE:
        _CACHE[key] = build(mm_dtype_name)
    return _CACHE[key]


def _run(inputs, mm_dtype_name="float16", trace=False):
    from concourse.bass_utils import run_bass_kernel_spmd

    import ml_dtypes

    np_mm = {"bfloat16": ml_dtypes.bfloat16, "float16": np.float16}.get(
        mm_dtype_name, np.float32
    )
    bf16 = np_mm != np.float32
    h = np.ascontiguousarray(np.asarray(inputs["h"], dtype=np.float32))
    if bf16:
        h = np.ascontiguousarray(h.T.astype(np_mm))  # pre-transposed [IN, B]
    gate_logits = np.asarray(inputs["gate_logits"], dtype=np.float64)
    W1 = np.asarray(inputs["W1"], dtype=np.float32)
    b1 = np.asarray(inputs["b1"], dtype=np.float32)
    W2 = np.asarray(inputs["W2"], dtype=np.float32)
    b2 = np.asarray(inputs["b2"], dtype=np.float32)

    # gate: softmax over E (uniform for zero logits); fold into W2/b2 per expert
    z = np.exp(gate_logits - gate_logits.max())
    probs = (z / z.sum()).astype(np.float32)

    in_maps = []
    for e in range(NCORES):
        w1_e = np.ascontiguousarray(W1[e].astype(np_mm))         # [IN, H]
        b1t_e = np.ascontiguousarray(b1[e].reshape(MC1, P).T)    # [P, MC1]
        w2_e = np.ascontiguousarray((W2[e] * probs[e]).astype(np_mm))  # [H, D]
        b2b_e = np.ascontiguousarray(
            np.broadcast_to(b2[e] * probs[e], (P, D))
        )
        in_maps.append(
            {("ht" if bf16 else "h"): h, "w1": w1_e, "b1t": b1t_e,
             "w2": w2_e, "b2b": b2b_e}
        )

    nc = _get_nc(mm_dtype_name)
    res = run_bass_kernel_spmd(nc, in_maps, list(range(NCORES)), trace=trace)

    # Reassemble. Chunks: tiles 0..NBT-2 are one 512-row RS each (64 rows per
    # core); the final tile is two 256-row RS (32 rows per core). Core r's
    # shard of a chunk starting at global row g0 with rows_per_core rpc lands
    # at final[g0 + r*rpc : g0 + (r+1)*rpc].
    chunks = []          # (global_row0, out_row0, rows_per_core)
    out_pos = 0
    for t in range(NBT):
        nhalves = 2 if t == NBT - 1 else 1
        rows = BT // nhalves
        for half in range(nhalves):
            rpc = rows // NCORES
            chunks.append((t * BT + half * rows, out_pos, rpc))
            out_pos += rpc
    final = np.empty((B, D), dtype=np.float32)
    for r in range(NCORES):
        o = res.results[r]["out"]
        for g0, o0, rpc in chunks:
            final[g0 + r * rpc: g0 + (r + 1) * rpc] = o[o0: o0 + rpc]
    return final, res


def kernel(**inputs):
    mm_dtype_name = os.environ.get("MOE_MM_DTYPE", "float16")
    final, _ = _run(inputs, mm_dtype_name=mm_dtype_name, trace=False)
    return final



# revision 5
# speedup vs baseline: 1.1144x; 1.1144x over previous
# Dense-MoE (all experts active) Trainium2 kernel, expert-parallel over 8
# NeuronCores. Each core computes its expert's 2-layer MLP over all tokens:
#   fe_e = gelu(h @ W1[e] + b1[e]) @ (probs[e] * W2[e])
# then a chunked fp16 ReduceScatter(add) across the 8 cores sums the expert
# contributions; the host reassembles the full [B, D] output and adds the
# (token-independent) bias term sum_e probs[e]*b2[e].
#
# Layout: activations stay transposed on-chip; L2 output is [D, tokens].
#   hT   [IN, B]   fp16, pre-transposed on the host
#   hidT [H, blk]  = (W1 block).T @ hT per 128-row chunk, gelu+b1 via ACT
#   feT  [D, blk]  = (W2 block).T @ hidT accumulated over H chunks
#
# The point of this structure vs. the naive loop order: every stationary
# 128x128 weight tile streams 4x512 moving columns (ldweights=False on the
# trailing 3 matmuls), so the PE array's ~128-cycle weight self-load is paid
# once per 2048 columns instead of once per 512. PSUM banks alternate in
# groups of 4 between consecutive passes so the Gelu/drain engines never
# block the next pass's matmuls.
import os
import sys

sys.path.insert(0, "/opt/trn_rl_repo")

import numpy as np

import concourse.mybir as mybir
from concourse import bacc, tile

B, E, IN, H, D = 4096, 8, 1024, 2048, 1024
NCORES = 8
P = 128
NBLK = 2                  # token blocks; phases L1(b) -> L2(b) run serially
BLK = B // NBLK           # 2048 tokens per block
NBANK = BLK // 512        # 4 PSUM banks per pass (512 fp32 cols each)
KC1 = IN // P             # 8 contraction chunks, layer 1
MC1 = H // P              # 16 H chunks (layer-1 output rows)
DC2 = D // P              # 8 D chunks (layer-2 output rows)
RSR = P // NCORES         # 16 rows each core receives per ReduceScatter

F32 = mybir.dt.float32

_CACHE = {}


def build(mm_dtype_name="float16", rs_dtype_name="float16"):
    mm_dt = getattr(mybir.dt, mm_dtype_name)
    rs_dt = getattr(mybir.dt, rs_dtype_name)
    assert mybir.dt.size(mm_dt) == 2, "matmul path requires a 16-bit dtype"
    nc = bacc.Bacc("TRN2", target_bir_lowering=False)

    hT = nc.declare_dram_parameter("ht", [IN, B], mm_dt, isOutput=False)
    w1 = nc.declare_dram_parameter("w1", [IN, H], mm_dt, isOutput=False)
    b1t = nc.declare_dram_parameter("b1t", [P, MC1], F32, isOutput=False)
    w2 = nc.declare_dram_parameter("w2", [H, D], mm_dt, isOutput=False)
    # out rows: 8 Dc chunks x 16 rows of this core's RS shard; cols: tokens
    out = nc.declare_dram_parameter("out", [P, B], rs_dt, isOutput=True)

    with tile.TileContext(nc) as tc:
        with (
            tc.tile_pool(name="weights", bufs=1) as wpool,
            tc.tile_pool(name="consts", bufs=1) as cpool,
            tc.tile_pool(name="ht", bufs=1) as ht_pool,
            tc.tile_pool(name="hid", bufs=MC1) as hid_pool,
            tc.tile_pool(name="fe", bufs=2) as fe_pool,
            tc.tile_pool(name="ps", bufs=2 * NBANK, space="PSUM") as ps_pool,
            tc.tile_pool(name="dram", bufs=3, space="DRAM") as dram_pool,
        ):
            # --- weight / input DMAs, ordered for fastest L1 start: the
            # first m-pass needs all 8 (w1 slab, ht slab) pairs of block 0,
            # so those stream first; w2 and block 1's ht follow (needed
            # ~100us and ~230us later respectively).
            w1_sb = []
            ht_sb = [[None] * KC1 for _ in range(NBLK)]
            for k in range(KC1):
                t_ = wpool.tile([P, H], mm_dt, tag=f"w1_{k}")
                nc.sync.dma_start(t_[:], w1[k * P:(k + 1) * P, :])
                w1_sb.append(t_)
                t_ = ht_pool.tile([P, BLK], mm_dt, tag=f"ht_0_{k}")
                nc.sync.dma_start(t_[:], hT[k * P:(k + 1) * P, 0:BLK])
                ht_sb[0][k] = t_
            b1_sb = cpool.tile([P, MC1], F32, tag="b1")
            nc.sync.dma_start(b1_sb[:], b1t[:])
            w2_sb = []
            for hc in range(MC1):
                t_ = wpool.tile([P, D], mm_dt, tag=f"w2_{hc}")
                nc.sync.dma_start(t_[:], w2[hc * P:(hc + 1) * P, :])
                w2_sb.append(t_)
            for b in range(1, NBLK):
                for k in range(KC1):
                    t_ = ht_pool.tile([P, BLK], mm_dt, tag=f"ht_{b}_{k}")
                    nc.sync.dma_start(
                        t_[:], hT[k * P:(k + 1) * P, b * BLK:(b + 1) * BLK]
                    )
                    ht_sb[b][k] = t_

            for b in range(NBLK):
                # --- L1 phase: hidT[m] = gelu((W1 block m).T @ hT + b1[m]) ---
                hid_sb = []
                for m in range(MC1):
                    banks = [
                        ps_pool.tile([P, 512], F32, tag="ps", name=f"ps{j}")
                        for j in range(NBANK)
                    ]
                    for k in range(KC1):
                        for j in range(NBANK):
                            mi = nc.tensor.matmul(
                                banks[j][:],
                                w1_sb[k][:, m * P:(m + 1) * P],
                                ht_sb[b][k][:, j * 512:(j + 1) * 512],
                                start=(k == 0),
                                stop=(k == KC1 - 1),
                            )
                            if j > 0:
                                mi.ins.ldweights = False
                    hm = hid_pool.tile([P, BLK], mm_dt, tag="hid")
                    for j in range(NBANK):
                        nc.scalar.activation(
                            hm[:, j * 512:(j + 1) * 512],
                            banks[j][:],
                            mybir.ActivationFunctionType.Gelu,
                            bias=b1_sb[:, m:m + 1],
                            scale=1.0,
                        )
                    hid_sb.append(hm)

                # --- L2 phase + chunked ReduceScatter per Dc ---
                for dc in range(DC2):
                    banks = [
                        ps_pool.tile([P, 512], F32, tag="ps", name=f"ps{j}")
                        for j in range(NBANK)
                    ]
                    for hc in range(MC1):
                        for j in range(NBANK):
                            mi = nc.tensor.matmul(
                                banks[j][:],
                                w2_sb[hc][:, dc * P:(dc + 1) * P],
                                hid_sb[hc][:, j * 512:(j + 1) * 512],
                                start=(hc == 0),
                                stop=(hc == MC1 - 1),
                            )
                            if j > 0:
                                mi.ins.ldweights = False
                    fe_sb = fe_pool.tile([P, BLK], rs_dt, tag="fe")
                    for j in range(NBANK):
                        nc.scalar.activation(
                            fe_sb[:, j * 512:(j + 1) * 512],
                            banks[j][:],
                            mybir.ActivationFunctionType.Copy,
                        )
                    fe_chunk = dram_pool.tile([P, BLK], rs_dt, tag="fe_dram")
                    nc.sync.dma_start(fe_chunk[:], fe_sb[:])
                    rs_chunk = dram_pool.tile([RSR, BLK], rs_dt, tag="rs_dram")
                    nc.gpsimd.collective_compute(
                        "ReduceScatter",
                        mybir.AluOpType.add,
                        replica_groups=[list(range(NCORES))],
                        ins=[fe_chunk[:]],
                        outs=[rs_chunk[:]],
                    )
                    nc.sync.dma_start(
                        out[dc * RSR:(dc + 1) * RSR, b * BLK:(b + 1) * BLK],
                        rs_chunk[:],
                    )

    nc.finalize()
    return nc


def _get_nc(mm_dtype_name, rs_dtype_name):
    key = (mm_dtype_name, rs_dtype_name)
    if key not in _CACHE:
        _CACHE[key] = build(mm_dtype_name, rs_dtype_name)
    return _CACHE[key]


def _run(inputs, mm_dtype_name="float16", trace=False):
    from concourse.bass_utils import run_bass_kernel_spmd

    import ml_dtypes

    rs_dtype_name = os.environ.get("MOE_RS_DTYPE", "float16")
    np_mm = {"bfloat16": ml_dtypes.bfloat16, "float16": np.float16}[
        mm_dtype_name
    ]
    h = np.ascontiguousarray(np.asarray(inputs["h"], dtype=np.float32))
    hT = np.ascontiguousarray(h.T.astype(np_mm))  # [IN, B]
    gate_logits = np.asarray(inputs["gate_logits"], dtype=np.float64)
    W1 = np.asarray(inputs["W1"], dtype=np.float32)
    b1 = np.asarray(inputs["b1"], dtype=np.float32)
    W2 = np.asarray(inputs["W2"], dtype=np.float32)
    b2 = np.asarray(inputs["b2"], dtype=np.float32)

    # gate: softmax over E (uniform for zero logits); fold into W2 per expert
    z = np.exp(gate_logits - gate_logits.max())
    probs = (z / z.sum()).astype(np.float32)

    in_maps = []
    for e in range(NCORES):
        in_maps.append({
            "ht": hT,
            "w1": np.ascontiguousarray(W1[e].astype(np_mm)),          # [IN,H]
            "b1t": np.ascontiguousarray(b1[e].reshape(MC1, P).T),     # [P,MC1]
            "w2": np.ascontiguousarray((W2[e] * probs[e]).astype(np_mm)),
        })

    nc = _get_nc(mm_dtype_name, rs_dtype_name)
    res = run_bass_kernel_spmd(nc, in_maps, list(range(NCORES)), trace=trace)

    # Reassemble: core r's out row (dc*RSR + i) is global D row
    # dc*P + r*RSR + i; columns are tokens in natural order.
    feT = np.empty((D, B), dtype=np.float32)
    for r in range(NCORES):
        o = np.asarray(res.results[r]["out"], dtype=np.float32)
        for dc in range(DC2):
            feT[dc * P + r * RSR: dc * P + (r + 1) * RSR, :] = (
                o[dc * RSR:(dc + 1) * RSR, :]
            )
    final = feT.T.copy()
    final += (probs @ b2)[None, :]  # token-independent bias term
    return final, res


def kernel(**inputs):
    mm_dtype_name = os.environ.get("MOE_MM_DTYPE", "float16")
    final, _ = _run(inputs, mm_dtype_name=mm_dtype_name, trace=False)
    return final
